# revision 1
# baseline (speedup 1.0000x reference)
"""Additive (Bahdanau) attention kernel for one TRN2 chip (8 NeuronCores).

Computes, for query (B,D), keys (B,S,D), mask (B,S), W1 (A,D), W2 (A,D), v (A,):
    scores[b,s] = v . tanh(W1 @ query[b] + W2 @ keys[b,s])
    out = softmax(scores - 1e30 * ~mask, axis=-1)

Sharding: data-parallel over batch B across the 8 cores (4 batches/core);
W1/W2/v replicated. No collectives needed; per-core outputs are concatenated
on the host.

Per-core device kernel (main matmuls in float32r at full PE rate):
  - w1q[a,b]    = W1 @ q_b              (tiny matmul, a on partitions)
  - per (s-tile of 512, b), per a-block j of 128:
        psum[a,s] += W2T_blk.T @ keysT_tile    (8 k-blocks, PE)
        comb = tanh(psum + w1q_j[:,b])         (ScalarE, per-partition bias)
        acc  += v_j * comb                     (VectorE mul+add chain)
    last add writes acc in f32r; a one-hot ones matmul per (s-tile, b)
    partition-reduces acc into row b of a shared [4, 512] psum tile
  - scores[:, s-tile] = sc_psum + maskadd      (additive -1e30 mask)
  - running row-max per s-tile; softmax tail: exp(+bias, accum sum) ->
    recip -> scale

Weights are stored per a-block (contiguous 512 KB DRAM blocks, one SBUF tile
each) so Tile's per-tile dependency tracking lets the j-th matmul group start
as soon as its own block has landed. Keys stream on the sync DMA queue,
weights on the scalar queue, ordered to stay ahead of PE consumption; a short
burst of junk matmuls warms the PE HAM clock gate during the initial DMA wait.
"""

import numpy as np

B, S, D, A = 32, 2048, 1024, 1024
NCORES = 8
BL = B // NCORES  # 4 batches per core
ST = 512          # s-tile width
NST = S // ST     # 4 s-tiles per batch
KB = D // 128     # 8 contraction blocks
JB = A // 128     # 8 attn-dim blocks
MASK_NEG = 1e30

_cache = {}


def _build_nc():
    from contextlib import ExitStack

    import concourse.tile as tile
    from concourse import bacc, mybir

    f32 = mybir.dt.float32
    f32r = mybir.dt.float32r
    bf16 = mybir.dt.bfloat16
    Tanh = mybir.ActivationFunctionType.Tanh
    Exp = mybir.ActivationFunctionType.Exp

    nc = bacc.Bacc(
        "TRN2",
        target_bir_lowering=False,
        debug=False,
        enable_asserts=False,
        num_devices=NCORES,
    )

    keysT = nc.dram_tensor("keysT", [D, BL, S], f32r, kind="ExternalInput").ap()
    # per-a-block weight blocks: [j, p, k*128+ai] = W[j*128+ai, k*128+p]
    w2t = nc.dram_tensor("w2t", [JB, 128, KB * 128], f32r, kind="ExternalInput").ap()
    w1t = nc.dram_tensor("w1t", [JB, 128, KB * 128], bf16, kind="ExternalInput").ap()
    qT = nc.dram_tensor("qT", [128, KB, BL], f32r, kind="ExternalInput").ap()
    qTb = nc.dram_tensor("qTb", [128, KB, BL], bf16, kind="ExternalInput").ap()
    # vcol[p, j] = v[j*128+p] — per-partition scalar for the DVE multiply
    vcol = nc.dram_tensor("vcol", [128, JB], f32, kind="ExternalInput").ap()
    # onesz[p, b, c] = 1 if b == c else 0 — one-hot ones column per batch so
    # each batch's partition-reduce lands in its own psum row
    onesz = nc.dram_tensor("onesz", [128, BL * BL], f32r, kind="ExternalInput").ap()
    maskadd = nc.dram_tensor("maskadd", [BL, S], f32, kind="ExternalInput").ap()
    out = nc.dram_tensor("out", [BL, S], f32, kind="ExternalOutput").ap()

    keysT_r = keysT.rearrange("(k p) b s -> p k b s", p=128)

    with tile.TileContext(nc) as tc, ExitStack() as ctx:
        singles = ctx.enter_context(tc.tile_pool(name="singles", bufs=1))
        keysp = ctx.enter_context(tc.tile_pool(name="keys", bufs=2))
        combp = ctx.enter_context(tc.tile_pool(name="comb", bufs=3))
        accp = ctx.enter_context(tc.tile_pool(name="acc", bufs=3))
        accrp = ctx.enter_context(tc.tile_pool(name="accr", bufs=2))
        tmpp = ctx.enter_context(tc.tile_pool(name="tmp", bufs=3))
        psmain = ctx.enter_context(tc.tile_pool(name="psmain", bufs=2, space="PSUM"))
        psvdot = ctx.enter_context(tc.tile_pool(name="psvdot", bufs=2, space="PSUM"))
        psw1q = ctx.enter_context(tc.tile_pool(name="psw1q", bufs=2, space="PSUM"))

        # --- staged input DMAs ---------------------------------------------
        # sync HWDGE queue (fast): q for warmup, W2 j=0, first keys tile,
        # then W2 j=1..7 ahead of the keys stream
        q_sb = singles.tile([128, KB, BL], f32r)
        nc.sync.dma_start(q_sb[:], qT)
        w2_sbj = [singles.tile([128, KB * 128], f32r, name=f"w2_sb{j}") for j in range(JB)]
        nc.sync.dma_start(w2_sbj[0][:], w2t[0])
        kt0 = keysp.tile([128, KB, ST], f32r)
        nc.sync.dma_start(kt0[:], keysT_r[:, :, 0, 0:ST])
        for j in range(1, JB):
            nc.sync.dma_start(w2_sbj[j][:], w2t[j])

        # scalar HWDGE queue (slower, starts later): everything the tanh-bias
        # path needs — W1 in bf16 halves its bytes so w1q_j7 beats its deadline
        v_sb = singles.tile([128, JB], f32)
        nc.scalar.dma_start(v_sb[:], vcol)
        o_one = singles.tile([128, BL * BL], f32r)
        nc.scalar.dma_start(o_one[:], onesz)
        qb_sb = singles.tile([128, KB, BL], bf16)
        nc.scalar.dma_start(qb_sb[:], qTb)
        w1_sbj = [singles.tile([128, KB * 128], bf16, name=f"w1_sb{j}") for j in range(JB)]
        for j in range(JB):
            nc.scalar.dma_start(w1_sbj[j][:], w1t[j])
        ma_sb = singles.tile([BL, S], f32)
        nc.scalar.dma_start(ma_sb[:], maskadd)

        scores = singles.tile([BL, S], f32)
        w1qj = [singles.tile([128, BL], f32, name=f"w1q{j}") for j in range(JB)]

        # preload the exp_and_others ACT table set (covers Tanh+Exp+Copy)
        # during the initial DMA wait instead of stalling the first tanh
        dummy_act = singles.tile([128, 1], f32)
        nc.scalar.activation(dummy_act[:], v_sb[:, 0:1], Tanh)

        # HAM warmup: junk matmuls on the first-arriving input keep the PE
        # busy through the clock-gate window while the real data streams in
        warm_ps = psw1q.tile([BL, KB * BL], f32)
        q_flat = q_sb[:].rearrange("p k b -> p (k b)")
        for w in range(260):
            nc.tensor.matmul(
                warm_ps[:],
                lhsT=q_sb[:, w % KB, :],
                rhs=q_flat,
                start=(w == 0),
                stop=(w == 259),
            )

        def emit_w1q(j):
            # w1q_j[a, b] = sum_d W1[a, d] q[b, d], a on partitions
            wq_ps = psw1q.tile([128, BL], f32)
            for k in range(KB):
                nc.tensor.matmul(
                    wq_ps[:],
                    lhsT=w1_sbj[j][:, k * 128 : (k + 1) * 128],
                    rhs=qb_sb[:, k, :],
                    start=(k == 0),
                    stop=(k == KB - 1),
                )
            nc.scalar.copy(w1qj[j][:], wq_ps[:])

        # --- main loop ------------------------------------------------------
        # the one-hot partition-reduce matmul for (st, b) is emitted two main
        # j-groups later so the in-order PE never waits on the tanh+DVE chain
        groups_done = 0
        ones_queue = []  # (sc_ps, accr, b, st, group_when_ready)

        def flush_ones(min_age):
            while ones_queue and groups_done - ones_queue[0][4] >= min_age:
                sc_ps_q, accr_q, b_q, st_q, _ = ones_queue.pop(0)
                nc.tensor.matmul(
                    sc_ps_q[:],
                    lhsT=o_one[:, b_q * BL : (b_q + 1) * BL],
                    rhs=accr_q[:],
                    start=(b_q == 0),
                    stop=(b_q == BL - 1),
                )
                if b_q == BL - 1:
                    nc.vector.tensor_add(
                        scores[:, st_q * ST : (st_q + 1) * ST],
                        sc_ps_q[:, :],
                        ma_sb[:, st_q * ST : (st_q + 1) * ST],
                    )

        sc_tiles = [psvdot.tile([BL, ST], f32, name=f"sc_ps{st}", tag="sc_ps") for st in range(NST)]
        for st in range(NST):
            sc_ps = sc_tiles[st]
            for b in range(BL):
                if st == 0 and b == 0:
                    kt = kt0
                else:
                    kt = keysp.tile([128, KB, ST], f32r)
                    nc.sync.dma_start(
                        kt[:], keysT_r[:, :, b, st * ST : (st + 1) * ST]
                    )
                acc = accp.tile([128, ST], f32)
                accr = accrp.tile([128, ST], f32r)
                for j in range(JB):
                    if st == 0 and b == 0:
                        # just-in-time w1q: emitted into the PE stream right
                        # before the main group that its tanh will need
                        emit_w1q(j)
                    ps = psmain.tile([128, ST], f32)
                    for k in range(KB):
                        nc.tensor.matmul(
                            ps[:],
                            lhsT=w2_sbj[j][:, k * 128 : (k + 1) * 128],
                            rhs=kt[:, k, :],
                            start=(k == 0),
                            stop=(k == KB - 1),
                        )
                    groups_done += 1
                    flush_ones(2)
                    comb = combp.tile([128, ST], f32)
                    nc.scalar.activation(
                        comb[:], ps[:], Tanh, bias=w1qj[j][:, b : b + 1]
                    )
                    # acc += v_j * comb on VectorE
                    if j == 0:
                        nc.vector.tensor_scalar_mul(acc[:], comb[:], v_sb[:, 0:1])
                    else:
                        tmp = tmpp.tile([128, ST], f32)
                        nc.vector.tensor_scalar_mul(
                            tmp[:], comb[:], v_sb[:, j : j + 1]
                        )
                        if j == JB - 1:
                            nc.vector.tensor_add(accr[:], acc[:], tmp[:])
                        else:
                            nc.vector.tensor_add(acc[:], acc[:], tmp[:])
                ones_queue.append((sc_ps, accr, b, st, groups_done))
        flush_ones(0)

        # --- masked softmax over S for the 4 batch rows ---------------------
        # scores are bounded by sum|v| (~27), so exp cannot overflow f32 and
        # the max subtraction is unnecessary; masked entries are exp(-1e30)=0.
        # Two half-width chunks pipeline ACT (exp) against DVE (sum) and the
        # output DMA against the second scale.
        H = S // 2
        e_sb = singles.tile([BL, S], f32)
        sums = singles.tile([BL, 2], f32)
        for h in range(2):
            nc.scalar.activation(
                e_sb[:, h * H : (h + 1) * H], scores[:, h * H : (h + 1) * H], Exp
            )
            nc.vector.reduce_sum(
                sums[:, h : h + 1],
                e_sb[:, h * H : (h + 1) * H],
                axis=mybir.AxisListType.X,
            )
        sm = singles.tile([BL, 1], f32)
        nc.vector.tensor_add(sm[:], sums[:, 0:1], sums[:, 1:2])
        rs = singles.tile([BL, 1], f32)
        nc.vector.reciprocal(rs[:], sm[:])
        o_sb = singles.tile([BL, S], f32)
        for h in range(2):
            nc.vector.tensor_scalar_mul(
                o_sb[:, h * H : (h + 1) * H], e_sb[:, h * H : (h + 1) * H], rs[:, 0:1]
            )
            nc.sync.dma_start(out[:, h * H : (h + 1) * H], o_sb[:, h * H : (h + 1) * H])

    nc.compile()
    return nc


def _get_nc():
    if "nc" not in _cache:
        _cache["nc"] = _build_nc()
    return _cache["nc"]


def _weight_blocks(W):
    # [j, p, k*128+ai] = W[j*128+ai, k*128+p]
    return np.ascontiguousarray(
        W.reshape(JB, 128, KB, 128).transpose(0, 3, 2, 1).reshape(JB, 128, KB * 128)
    )


def _make_in_maps(query, keys, mask, W1, W2, v):
    query = np.asarray(query, dtype=np.float32)
    keys = np.asarray(keys, dtype=np.float32)
    mask = np.asarray(mask)
    W1 = np.asarray(W1, dtype=np.float32)
    W2 = np.asarray(W2, dtype=np.float32)
    v = np.asarray(v, dtype=np.float32)

    import ml_dtypes

    w2t = _weight_blocks(W2)
    w1t = _weight_blocks(W1).astype(ml_dtypes.bfloat16)
    vcol = np.ascontiguousarray(v.reshape(JB, 128).T)  # [p, j]
    onesz = np.zeros((128, BL, BL), dtype=np.float32)
    for b in range(BL):
        onesz[:, b, b] = 1.0
    onesz = np.ascontiguousarray(onesz.reshape(128, BL * BL))

    in_maps = []
    for c in range(NCORES):
        sl = slice(c * BL, (c + 1) * BL)
        keysT_c = np.ascontiguousarray(keys[sl].transpose(2, 0, 1))  # (D, BL, S)
        qT_c = np.ascontiguousarray(
            query[sl].T.reshape(KB, 128, BL).transpose(1, 0, 2)
        )  # (128, KB, BL)
        qTb_c = qT_c.astype(ml_dtypes.bfloat16)
        maskadd_c = np.where(mask[sl], 0.0, -MASK_NEG).astype(np.float32)
        in_maps.append(
            {
                "keysT": keysT_c,
                "w2t": w2t,
                "w1t": w1t,
                "qT": qT_c,
                "qTb": qTb_c,
                "vcol": vcol,
                "onesz": onesz,
                "maskadd": maskadd_c,
            }
        )
    return in_maps


def kernel(query, keys, mask, W1, W2, v):
    from concourse.bass_utils import run_bass_kernel_spmd

    nc = _get_nc()
    in_maps = _make_in_maps(query, keys, mask, W1, W2, v)
    res = run_bass_kernel_spmd(nc, in_maps, core_ids=list(range(NCORES)))
    _cache["last_results"] = res
    out = np.concatenate([res.results[i]["out"] for i in range(NCORES)], axis=0)
    return out.astype(np.float32)



# revision 10
# speedup vs baseline: 1.0718x; 1.0718x over previous
"""Additive (Bahdanau) attention kernel for one TRN2 chip (8 NeuronCores).

Computes, for query (B,D), keys (B,S,D), mask (B,S), W1 (A,D), W2 (A,D), v (A,):
    scores[b,s] = v . tanh(W1 @ query[b] + W2 @ keys[b,s])
    out = softmax(scores - 1e30 * ~mask, axis=-1)

Sharding: data-parallel over batch B across the 8 cores (4 batches/core);
W1/W2/v replicated. No collectives needed; per-core outputs are concatenated
on the host.

Per-core device kernel (main matmuls in fp8 e4m3 DoubleRow at 0.5 cyc/row):
  - keys are split host-side as k = e4m3(k) + e4m3(k - e4m3(k)) (hi/lo), so
    the fp8 matmul keeps ~12-bit effective keys precision; W2 is single
    e4m3 (measured end-to-end rel err 0.0163 < 2e-2 gate)
  - w1q[a,b]    = W1 @ q_b              (tiny matmul, a on partitions)
  - per (s-tile of 512, b), per a-block j of 128:
        psum[a,s] += W2T_pair.T @ khi_pair + W2T_pair.T @ klo_pair
                     (4 k-pair DoubleRow matmuls x {hi,lo}, each contracting
                      256 dims in 256 cycles; hi/lo share the stationary)
        comb = tanh(psum + w1q_j[:,b])         (ScalarE, per-partition bias)
        acc  += v_j * comb                     (VectorE mul+add chain)
    last add writes acc in f32r; a one-hot ones matmul per (s-tile, b)
    partition-reduces acc into row b of a shared [4, 512] psum tile
  - scores[:, s-tile] = sc_psum + maskadd      (additive -1e30 mask)
  - running row-max per s-tile; softmax tail: exp(+bias, accum sum) ->
    recip -> scale

Weights are stored per a-block (contiguous 512 KB DRAM blocks, one SBUF tile
each) so Tile's per-tile dependency tracking lets the j-th matmul group start
as soon as its own block has landed. Keys stream on the sync DMA queue,
weights on the scalar queue, ordered to stay ahead of PE consumption; a short
burst of junk matmuls warms the PE HAM clock gate during the initial DMA wait.
"""

import numpy as np

B, S, D, A = 32, 2048, 1024, 1024
NCORES = 8
BL = B // NCORES  # 4 batches per core
ST = 512          # s-tile width
NST = S // ST     # 4 s-tiles per batch
KB = D // 128     # 8 contraction blocks
KP = KB // 2      # 4 DoubleRow contraction pairs
JB = A // 128     # 8 attn-dim blocks
MASK_NEG = 1e30

_cache = {}


def _build_nc():
    from contextlib import ExitStack

    import concourse.tile as tile
    from concourse import bacc, mybir

    f32 = mybir.dt.float32
    f32r = mybir.dt.float32r
    bf16 = mybir.dt.bfloat16
    f8 = mybir.dt.float8e4
    DR = mybir.MatmulPerfMode.DoubleRow
    Tanh = mybir.ActivationFunctionType.Tanh
    Exp = mybir.ActivationFunctionType.Exp

    nc = bacc.Bacc(
        "TRN2",
        target_bir_lowering=False,
        debug=False,
        enable_asserts=False,
        num_devices=NCORES,
    )

    # hi/lo e4m3 split of keys: keysT8[i, d, b, s], i=0 hi, i=1 lo
    keysT8 = nc.dram_tensor("keysT8", [2, D, BL, S], f8, kind="ExternalInput").ap()
    # per-a-block weight blocks: [j, p, k*128+ai] = W[j*128+ai, k*128+p]
    w2t = nc.dram_tensor("w2t", [JB, 128, KB * 128], f8, kind="ExternalInput").ap()
    w1t = nc.dram_tensor("w1t", [JB, 128, KB * 128], bf16, kind="ExternalInput").ap()
    qT = nc.dram_tensor("qT", [128, KB, BL], f32r, kind="ExternalInput").ap()
    qTb = nc.dram_tensor("qTb", [128, KB, BL], bf16, kind="ExternalInput").ap()
    # vcol[p, j] = v[j*128+p] — per-partition scalar for the DVE multiply
    vcol = nc.dram_tensor("vcol", [128, JB], f32, kind="ExternalInput").ap()
    # onesz[p, b, c] = 1 if b == c else 0 — one-hot ones column per batch so
    # each batch's partition-reduce lands in its own psum row
    onesz = nc.dram_tensor("onesz", [128, BL * BL], f32r, kind="ExternalInput").ap()
    maskadd = nc.dram_tensor("maskadd", [BL, S], f32, kind="ExternalInput").ap()
    out = nc.dram_tensor("out", [BL, S], f32, kind="ExternalOutput").ap()

    keysT_r = keysT8.rearrange("i (k p) b s -> p i k b s", p=128)

    with tile.TileContext(nc) as tc, ExitStack() as ctx:
        singles = ctx.enter_context(tc.tile_pool(name="singles", bufs=1))
        keysp = ctx.enter_context(tc.tile_pool(name="keys", bufs=2))
        combp = ctx.enter_context(tc.tile_pool(name="comb", bufs=3))
        accp = ctx.enter_context(tc.tile_pool(name="acc", bufs=3))
        accrp = ctx.enter_context(tc.tile_pool(name="accr", bufs=2))
        tmpp = ctx.enter_context(tc.tile_pool(name="tmp", bufs=3))
        psmain = ctx.enter_context(tc.tile_pool(name="psmain", bufs=2, space="PSUM"))
        psvdot = ctx.enter_context(tc.tile_pool(name="psvdot", bufs=2, space="PSUM"))
        psw1q = ctx.enter_context(tc.tile_pool(name="psw1q", bufs=2, space="PSUM"))

        # --- staged input DMAs ---------------------------------------------
        # sync HWDGE queue (fast): q for warmup, W2 j=0, first keys tile,
        # then W2 j=1..7 ahead of the keys stream
        q_sb = singles.tile([128, KB, BL], f32r)
        nc.sync.dma_start(q_sb[:], qT)
        w2_sbj = [singles.tile([128, KB * 128], f8, name=f"w2_sb{j}") for j in range(JB)]
        nc.sync.dma_start(w2_sbj[0][:], w2t[0])
        kt0 = keysp.tile([128, 2, KB, ST], f8)
        nc.sync.dma_start(kt0[:], keysT_r[:, :, :, 0, 0:ST])
        for j in range(1, JB):
            nc.sync.dma_start(w2_sbj[j][:], w2t[j])

        # scalar HWDGE queue (slower, starts later): everything the tanh-bias
        # path needs — W1 in bf16 halves its bytes so w1q_j7 beats its deadline
        v_sb = singles.tile([128, JB], f32)
        nc.scalar.dma_start(v_sb[:], vcol)
        o_one = singles.tile([128, BL * BL], f32r)
        nc.scalar.dma_start(o_one[:], onesz)
        qb_sb = singles.tile([128, KB, BL], bf16)
        nc.scalar.dma_start(qb_sb[:], qTb)
        w1_sbj = [singles.tile([128, KB * 128], bf16, name=f"w1_sb{j}") for j in range(JB)]
        for j in range(JB):
            nc.scalar.dma_start(w1_sbj[j][:], w1t[j])
        ma_sb = singles.tile([BL, S], f32)
        nc.scalar.dma_start(ma_sb[:], maskadd)

        scores = singles.tile([BL, S], f32)
        w1qj = [singles.tile([128, BL], f32, name=f"w1q{j}") for j in range(JB)]

        # preload the exp_and_others ACT table set (covers Tanh+Exp+Copy)
        # during the initial DMA wait instead of stalling the first tanh
        dummy_act = singles.tile([128, 1], f32)
        nc.scalar.activation(dummy_act[:], v_sb[:, 0:1], Tanh)

        # HAM warmup: junk matmuls on the first-arriving input keep the PE
        # busy through the clock-gate window while the real data streams in
        warm_ps = psw1q.tile([BL, KB * BL], f32)
        q_flat = q_sb[:].rearrange("p k b -> p (k b)")
        for w in range(260):
            nc.tensor.matmul(
                warm_ps[:],
                lhsT=q_sb[:, w % KB, :],
                rhs=q_flat,
                start=(w == 0),
                stop=(w == 259),
            )

        def emit_w1q(j):
            # w1q_j[a, b] = sum_d W1[a, d] q[b, d], a on partitions
            wq_ps = psw1q.tile([128, BL], f32)
            for k in range(KB):
                nc.tensor.matmul(
                    wq_ps[:],
                    lhsT=w1_sbj[j][:, k * 128 : (k + 1) * 128],
                    rhs=qb_sb[:, k, :],
                    start=(k == 0),
                    stop=(k == KB - 1),
                )
            nc.scalar.copy(w1qj[j][:], wq_ps[:])

        # --- main loop ------------------------------------------------------
        # the one-hot partition-reduce matmul for (st, b) is emitted two main
        # j-groups later so the in-order PE never waits on the tanh+DVE chain
        groups_done = 0
        ones_queue = []  # (sc_ps, accr, b, st, group_when_ready)

        def flush_ones(min_age):
            while ones_queue and groups_done - ones_queue[0][4] >= min_age:
                sc_ps_q, accr_q, b_q, st_q, _ = ones_queue.pop(0)
                nc.tensor.matmul(
                    sc_ps_q[:],
                    lhsT=o_one[:, b_q * BL : (b_q + 1) * BL],
                    rhs=accr_q[:],
                    start=(b_q == 0),
                    stop=(b_q == BL - 1),
                )
                if b_q == BL - 1:
                    nc.vector.tensor_add(
                        scores[:, st_q * ST : (st_q + 1) * ST],
                        sc_ps_q[:, :],
                        ma_sb[:, st_q * ST : (st_q + 1) * ST],
                    )

        sc_tiles = [psvdot.tile([BL, ST], f32, name=f"sc_ps{st}", tag="sc_ps") for st in range(NST)]
        for st in range(NST):
            sc_ps = sc_tiles[st]
            for b in range(BL):
                if st == 0 and b == 0:
                    kt = kt0
                else:
                    kt = keysp.tile([128, 2, KB, ST], f8)
                    nc.sync.dma_start(
                        kt[:], keysT_r[:, :, :, b, st * ST : (st + 1) * ST]
                    )
                acc = accp.tile([128, ST], f32)
                accr = accrp.tile([128, ST], f32r)
                for j in range(JB):
                    if st == 0 and b == 0:
                        # just-in-time w1q: emitted into the PE stream right
                        # before the main group that its tanh will need
                        emit_w1q(j)
                    ps = psmain.tile([128, ST], f32)
                    for kk in range(KP):
                        # DoubleRow pair: 256-dim contraction per instr; the
                        # hi and lo passes share the stationary weight pair
                        w3 = w2_sbj[j][:, kk * 256 : (kk + 1) * 256].rearrange(
                            "p (two m) -> p two m", two=2
                        )
                        nc.tensor.matmul(
                            ps[:],
                            lhsT=w3,
                            rhs=kt[:, 0, 2 * kk : 2 * kk + 2, :],
                            start=(kk == 0),
                            stop=False,
                            perf_mode=DR,
                        )
                        nc.tensor.matmul(
                            ps[:],
                            lhsT=w3,
                            rhs=kt[:, 1, 2 * kk : 2 * kk + 2, :],
                            start=False,
                            stop=(kk == KP - 1),
                            perf_mode=DR,
                        )
                    groups_done += 1
                    flush_ones(2)
                    comb = combp.tile([128, ST], f32)
                    nc.scalar.activation(
                        comb[:], ps[:], Tanh, bias=w1qj[j][:, b : b + 1]
                    )
                    # acc += v_j * comb on VectorE
                    if j == 0:
                        nc.vector.tensor_scalar_mul(acc[:], comb[:], v_sb[:, 0:1])
                    else:
                        tmp = tmpp.tile([128, ST], f32)
                        nc.vector.tensor_scalar_mul(
                            tmp[:], comb[:], v_sb[:, j : j + 1]
                        )
                        if j == JB - 1:
                            nc.vector.tensor_add(accr[:], acc[:], tmp[:])
                        else:
                            nc.vector.tensor_add(acc[:], acc[:], tmp[:])
                ones_queue.append((sc_ps, accr, b, st, groups_done))
        flush_ones(0)

        # --- masked softmax over S for the 4 batch rows ---------------------
        # scores are bounded by sum|v| (~27), so exp cannot overflow f32 and
        # the max subtraction is unnecessary; masked entries are exp(-1e30)=0.
        # Two half-width chunks pipeline ACT (exp) against DVE (sum) and the
        # output DMA against the second scale.
        H = S // 2
        e_sb = singles.tile([BL, S], f32)
        sums = singles.tile([BL, 2], f32)
        for h in range(2):
            nc.scalar.activation(
                e_sb[:, h * H : (h + 1) * H], scores[:, h * H : (h + 1) * H], Exp
            )
            nc.vector.reduce_sum(
                sums[:, h : h + 1],
                e_sb[:, h * H : (h + 1) * H],
                axis=mybir.AxisListType.X,
            )
        sm = singles.tile([BL, 1], f32)
        nc.vector.tensor_add(sm[:], sums[:, 0:1], sums[:, 1:2])
        rs = singles.tile([BL, 1], f32)
        nc.vector.reciprocal(rs[:], sm[:])
        o_sb = singles.tile([BL, S], f32)
        for h in range(2):
            nc.vector.tensor_scalar_mul(
                o_sb[:, h * H : (h + 1) * H], e_sb[:, h * H : (h + 1) * H], rs[:, 0:1]
            )
            nc.sync.dma_start(out[:, h * H : (h + 1) * H], o_sb[:, h * H : (h + 1) * H])

    nc.compile()
    return nc


def _get_nc():
    if "nc" not in _cache:
        _cache["nc"] = _build_nc()
    return _cache["nc"]


def _weight_blocks(W):
    # [j, p, k*128+ai] = W[j*128+ai, k*128+p]
    return np.ascontiguousarray(
        W.reshape(JB, 128, KB, 128).transpose(0, 3, 2, 1).reshape(JB, 128, KB * 128)
    )


def _make_in_maps(query, keys, mask, W1, W2, v):
    query = np.asarray(query, dtype=np.float32)
    keys = np.asarray(keys, dtype=np.float32)
    mask = np.asarray(mask)
    W1 = np.asarray(W1, dtype=np.float32)
    W2 = np.asarray(W2, dtype=np.float32)
    v = np.asarray(v, dtype=np.float32)

    import ml_dtypes

    f8np = ml_dtypes.float8_e4m3  # TRN float8e4 (max normal 240)
    w2t = _weight_blocks(W2).astype(f8np)
    w1t = _weight_blocks(W1).astype(ml_dtypes.bfloat16)
    vcol = np.ascontiguousarray(v.reshape(JB, 128).T)  # [p, j]
    onesz = np.zeros((128, BL, BL), dtype=np.float32)
    for b in range(BL):
        onesz[:, b, b] = 1.0
    onesz = np.ascontiguousarray(onesz.reshape(128, BL * BL))

    in_maps = []
    for c in range(NCORES):
        sl = slice(c * BL, (c + 1) * BL)
        keysT_c = np.ascontiguousarray(keys[sl].transpose(2, 0, 1))  # (D, BL, S)
        khi = keysT_c.astype(f8np)
        klo = (keysT_c - khi.astype(np.float32)).astype(f8np)
        keysT8_c = np.stack([khi, klo], axis=0)  # (2, D, BL, S)
        qT_c = np.ascontiguousarray(
            query[sl].T.reshape(KB, 128, BL).transpose(1, 0, 2)
        )  # (128, KB, BL)
        qTb_c = qT_c.astype(ml_dtypes.bfloat16)
        maskadd_c = np.where(mask[sl], 0.0, -MASK_NEG).astype(np.float32)
        in_maps.append(
            {
                "keysT8": keysT8_c,
                "w2t": w2t,
                "w1t": w1t,
                "qT": qT_c,
                "qTb": qTb_c,
                "vcol": vcol,
                "onesz": onesz,
                "maskadd": maskadd_c,
            }
        )
    return in_maps


def kernel(query, keys, mask, W1, W2, v):
    from concourse.bass_utils import run_bass_kernel_spmd

    nc = _get_nc()
    in_maps = _make_in_maps(query, keys, mask, W1, W2, v)
    res = run_bass_kernel_spmd(nc, in_maps, core_ids=list(range(NCORES)))
    _cache["last_results"] = res
    out = np.concatenate([res.results[i]["out"] for i in range(NCORES)], axis=0)
    return out.astype(np.float32)



# revision 17
# speedup vs baseline: 1.1011x; 1.0273x over previous
"""Additive (Bahdanau) attention kernel for one TRN2 chip (8 NeuronCores).

Computes, for query (B,D), keys (B,S,D), mask (B,S), W1 (A,D), W2 (A,D), v (A,):
    scores[b,s] = v . tanh(W1 @ query[b] + W2 @ keys[b,s])
    out = softmax(scores - 1e30 * ~mask, axis=-1)

Sharding: data-parallel over batch B across the 8 cores (4 batches/core);
W1/W2/v replicated. No collectives needed; per-core outputs are concatenated
on the host.

Per-core device kernel (main matmuls in fp8 e4m3 DoubleRow at 0.5 cyc/row):
  - keys are split host-side as k = e4m3(k) + e4m3(k - e4m3(k)) (hi/lo), so
    the fp8 matmul keeps ~12-bit effective keys precision; W2 is single
    e4m3 (measured end-to-end rel err 0.0163 < 2e-2 gate)
  - w1q[a,b]    = W1 @ q_b              (tiny matmul, a on partitions)
  - per (s-tile of 512, b), per a-block j of 128:
        psum[a,s] += W2T_pair.T @ khi_pair + W2T_pair.T @ klo_pair
                     (4 k-pair DoubleRow matmuls x {hi,lo}, each contracting
                      256 dims in 256 cycles; hi/lo share the stationary)
        comb = tanh(psum + w1q_j[:,b])         (ScalarE, per-partition bias)
        acc  += v_j * comb                     (VectorE mul+add chain)
    last add writes acc in f32r; a one-hot ones matmul per (s-tile, b)
    partition-reduces acc into row b of a shared [4, 512] psum tile
  - scores[:, s-tile] = sc_psum + maskadd      (additive -1e30 mask)
  - running row-max per s-tile; softmax tail: exp(+bias, accum sum) ->
    recip -> scale

Weights are stored per a-block (contiguous 512 KB DRAM blocks, one SBUF tile
each) so Tile's per-tile dependency tracking lets the j-th matmul group start
as soon as its own block has landed. Keys stream on the sync DMA queue,
weights on the scalar queue, ordered to stay ahead of PE consumption; a short
burst of junk matmuls warms the PE HAM clock gate during the initial DMA wait.
"""

import numpy as np

B, S, D, A = 32, 2048, 1024, 1024
NCORES = 8
BL = B // NCORES  # 4 batches per core
ST = 512          # s-tile width
NST = S // ST     # 4 s-tiles per batch
KB = D // 128     # 8 contraction blocks
KP = KB // 2      # 4 DoubleRow contraction pairs
JB = A // 128     # 8 attn-dim blocks
MASK_NEG = 1e30

_cache = {}


def _build_nc():
    from contextlib import ExitStack

    import concourse.tile as tile
    from concourse import bacc, mybir

    f32 = mybir.dt.float32
    f32r = mybir.dt.float32r
    bf16 = mybir.dt.bfloat16
    f8 = mybir.dt.float8e4
    DR = mybir.MatmulPerfMode.DoubleRowSwInterleave
    Mult = mybir.AluOpType.mult
    Add = mybir.AluOpType.add
    Tanh = mybir.ActivationFunctionType.Tanh
    Exp = mybir.ActivationFunctionType.Exp

    nc = bacc.Bacc(
        "TRN2",
        target_bir_lowering=False,
        debug=False,
        enable_asserts=False,
        num_devices=NCORES,
    )

    # hi/lo e4m3 split of keys: keysT8[i, d, b, s], i=0 hi, i=1 lo
    keysT8 = nc.dram_tensor("keysT8", [2, D, BL, S], f8, kind="ExternalInput").ap()
    # per-a-block weight blocks: [j, p, k*128+ai] = W[j*128+ai, k*128+p]
    w2t = nc.dram_tensor("w2t", [JB, 128, KB * 128], f8, kind="ExternalInput").ap()
    w1t = nc.dram_tensor("w1t", [JB, 128, KB * 128], bf16, kind="ExternalInput").ap()
    qT = nc.dram_tensor("qT", [128, KB, BL], f32r, kind="ExternalInput").ap()
    qTb = nc.dram_tensor("qTb", [128, KB, BL], bf16, kind="ExternalInput").ap()
    # vcol[p, j] = v[j*128+p] — per-partition scalar for the DVE multiply
    vcol = nc.dram_tensor("vcol", [128, JB], f32, kind="ExternalInput").ap()
    # onesz[p, b, c] = 1 if b == c else 0 — one-hot ones column per batch so
    # each batch's partition-reduce lands in its own psum row
    onesz = nc.dram_tensor("onesz", [128, BL * BL], f32r, kind="ExternalInput").ap()
    maskadd = nc.dram_tensor("maskadd", [BL, S], f32, kind="ExternalInput").ap()
    out = nc.dram_tensor("out", [BL, S], f32, kind="ExternalOutput").ap()

    keysT_r = keysT8.rearrange("i (k p) b s -> p i k b s", p=128)

    with tile.TileContext(nc) as tc, ExitStack() as ctx:
        singles = ctx.enter_context(tc.tile_pool(name="singles", bufs=1))
        keysp = ctx.enter_context(tc.tile_pool(name="keys", bufs=2))
        combp = ctx.enter_context(tc.tile_pool(name="comb", bufs=3))
        accp = ctx.enter_context(tc.tile_pool(name="acc", bufs=3))
        accrp = ctx.enter_context(tc.tile_pool(name="accr", bufs=2))
        tmpp = ctx.enter_context(tc.tile_pool(name="tmp", bufs=3))
        psmain = ctx.enter_context(tc.tile_pool(name="psmain", bufs=2, space="PSUM"))
        psvdot = ctx.enter_context(tc.tile_pool(name="psvdot", bufs=2, space="PSUM"))
        psw1q = ctx.enter_context(tc.tile_pool(name="psw1q", bufs=2, space="PSUM"))

        # --- staged input DMAs ---------------------------------------------
        # sync HWDGE queue (fast): q for warmup, W2 j=0, first keys tile,
        # then W2 j=1..7 ahead of the keys stream
        q_sb = singles.tile([128, KB, BL], f32r)
        nc.sync.dma_start(q_sb[:], qT)
        w2_sbj = [singles.tile([128, KB * 128], f8, name=f"w2_sb{j}") for j in range(JB)]
        nc.sync.dma_start(w2_sbj[0][:], w2t[0])
        kt0 = keysp.tile([128, 2, KB, ST], f8)
        nc.sync.dma_start(kt0[:], keysT_r[:, :, :, 0, 0:ST])
        for j in range(1, JB):
            nc.sync.dma_start(w2_sbj[j][:], w2t[j])

        # scalar HWDGE queue (slower, starts later): everything the tanh-bias
        # path needs — W1 in bf16 halves its bytes so w1q_j7 beats its deadline
        v_sb = singles.tile([128, JB], f32)
        nc.scalar.dma_start(v_sb[:], vcol)
        o_one = singles.tile([128, BL * BL], f32r)
        nc.scalar.dma_start(o_one[:], onesz)
        qb_sb = singles.tile([128, KB, BL], bf16)
        nc.scalar.dma_start(qb_sb[:], qTb)
        w1_sbj = [singles.tile([128, KB * 128], bf16, name=f"w1_sb{j}") for j in range(JB)]
        for j in range(JB):
            nc.scalar.dma_start(w1_sbj[j][:], w1t[j])
        ma_sb = singles.tile([BL, S], f32)
        nc.scalar.dma_start(ma_sb[:], maskadd)

        scores = singles.tile([BL, S], f32)
        w1qj = [singles.tile([128, BL], f32, name=f"w1q{j}") for j in range(JB)]

        # preload the exp_and_others ACT table set (covers Tanh+Exp+Copy)
        # during the initial DMA wait instead of stalling the first tanh
        dummy_act = singles.tile([128, 1], f32)
        nc.scalar.activation(dummy_act[:], v_sb[:, 0:1], Tanh)

        # HAM warmup: junk matmuls on the first-arriving input keep the PE
        # busy through the clock-gate window while the real data streams in
        warm_ps = psw1q.tile([BL, KB * BL], f32)
        q_flat = q_sb[:].rearrange("p k b -> p (k b)")
        NWARM = 200
        for w in range(NWARM):
            nc.tensor.matmul(
                warm_ps[:],
                lhsT=q_sb[:, w % KB, :],
                rhs=q_flat,
                start=(w == 0),
                stop=(w == NWARM - 1),
            )

        def emit_w1q(j):
            # w1q_j[a, b] = sum_d W1[a, d] q[b, d], a on partitions
            wq_ps = psw1q.tile([128, BL], f32)
            for k in range(KB):
                nc.tensor.matmul(
                    wq_ps[:],
                    lhsT=w1_sbj[j][:, k * 128 : (k + 1) * 128],
                    rhs=qb_sb[:, k, :],
                    start=(k == 0),
                    stop=(k == KB - 1),
                )
            nc.scalar.copy(w1qj[j][:], wq_ps[:])

        # --- main loop ------------------------------------------------------
        # the one-hot partition-reduce matmul for (st, b) is emitted two main
        # j-groups later so the in-order PE never waits on the tanh+DVE chain
        groups_done = 0
        ones_queue = []  # (sc_ps, accr, b, st, group_when_ready)

        def flush_ones(min_age):
            while ones_queue and groups_done - ones_queue[0][4] >= min_age:
                sc_ps_q, accr_q, b_q, st_q, _ = ones_queue.pop(0)
                nc.tensor.matmul(
                    sc_ps_q[:],
                    lhsT=o_one[:, b_q * BL : (b_q + 1) * BL],
                    rhs=accr_q[:],
                    start=(b_q == 0),
                    stop=(b_q == BL - 1),
                )
                if b_q == BL - 1:
                    nc.vector.tensor_add(
                        scores[:, st_q * ST : (st_q + 1) * ST],
                        sc_ps_q[:, :],
                        ma_sb[:, st_q * ST : (st_q + 1) * ST],
                    )

        sc_tiles = [psvdot.tile([BL, ST], f32, name=f"sc_ps{st}", tag="sc_ps") for st in range(NST)]
        for st in range(NST):
            sc_ps = sc_tiles[st]
            for b in range(BL):
                if st == 0 and b == 0:
                    kt = kt0
                else:
                    kt = keysp.tile([128, 2, KB, ST], f8)
                    nc.sync.dma_start(
                        kt[:], keysT_r[:, :, :, b, st * ST : (st + 1) * ST]
                    )
                acc = accp.tile([128, ST], f32)
                accr = accrp.tile([128, ST], f32r)
                for j in range(JB):
                    if st == 0 and b == 0:
                        # just-in-time w1q: emitted into the PE stream right
                        # before the main group that its tanh will need
                        emit_w1q(j)
                    ps = psmain.tile([128, ST], f32)
                    for kk in range(KP):
                        # DoubleRow pair: 256-dim contraction per instr; the
                        # hi and lo passes share the stationary weight pair
                        w3 = w2_sbj[j][:, kk * 256 : (kk + 1) * 256].rearrange(
                            "p (two m) -> p two m", two=2
                        )
                        nc.tensor.matmul(
                            ps[:],
                            lhsT=w3,
                            rhs=kt[:, 0, 2 * kk : 2 * kk + 2, :],
                            start=(kk == 0),
                            stop=False,
                            perf_mode=DR,
                        )
                        nc.tensor.matmul(
                            ps[:],
                            lhsT=w3,
                            rhs=kt[:, 1, 2 * kk : 2 * kk + 2, :],
                            start=False,
                            stop=(kk == KP - 1),
                            perf_mode=DR,
                        )
                    groups_done += 1
                    flush_ones(2)
                    comb = combp.tile([128, ST], f32)
                    nc.scalar.activation(
                        comb[:], ps[:], Tanh, bias=w1qj[j][:, b : b + 1]
                    )
                    # acc = (comb * v_j) + acc fused on VectorE
                    if j == 0:
                        nc.vector.tensor_scalar_mul(acc[:], comb[:], v_sb[:, 0:1])
                    elif j == JB - 1:
                        nc.vector.scalar_tensor_tensor(
                            accr[:], comb[:], v_sb[:, j : j + 1], acc[:], Mult, Add
                        )
                    else:
                        nc.vector.scalar_tensor_tensor(
                            acc[:], comb[:], v_sb[:, j : j + 1], acc[:], Mult, Add
                        )
                ones_queue.append((sc_ps, accr, b, st, groups_done))
        flush_ones(0)

        # --- masked softmax over S for the 4 batch rows ---------------------
        # scores are bounded by sum|v| (~27), so exp cannot overflow f32 and
        # the max subtraction is unnecessary; masked entries are exp(-1e30)=0.
        # Two half-width chunks pipeline ACT (exp) against DVE (sum) and the
        # output DMA against the second scale.
        H = S // 2
        e_sb = singles.tile([BL, S], f32)
        sums = singles.tile([BL, 2], f32)
        for h in range(2):
            nc.scalar.activation(
                e_sb[:, h * H : (h + 1) * H], scores[:, h * H : (h + 1) * H], Exp
            )
            nc.vector.reduce_sum(
                sums[:, h : h + 1],
                e_sb[:, h * H : (h + 1) * H],
                axis=mybir.AxisListType.X,
            )
        sm = singles.tile([BL, 1], f32)
        nc.vector.tensor_add(sm[:], sums[:, 0:1], sums[:, 1:2])
        rs = singles.tile([BL, 1], f32)
        nc.vector.reciprocal(rs[:], sm[:])
        o_sb = singles.tile([BL, S], f32)
        for h in range(2):
            nc.vector.tensor_scalar_mul(
                o_sb[:, h * H : (h + 1) * H], e_sb[:, h * H : (h + 1) * H], rs[:, 0:1]
            )
            nc.sync.dma_start(out[:, h * H : (h + 1) * H], o_sb[:, h * H : (h + 1) * H])

    nc.compile()
    return nc


def _get_nc():
    if "nc" not in _cache:
        _cache["nc"] = _build_nc()
    return _cache["nc"]


def _weight_blocks(W):
    # [j, p, k*128+ai] = W[j*128+ai, k*128+p]
    return np.ascontiguousarray(
        W.reshape(JB, 128, KB, 128).transpose(0, 3, 2, 1).reshape(JB, 128, KB * 128)
    )


def _weight_blocks_swil(W8):
    # DoubleRowSwInterleave storage for each k-pair's 256 columns:
    # [A127, B127, A126, B126, ..., A0, B0] where A/B are the logical
    # 128-column weight sets of k-blocks (2kk, 2kk+1)
    blk = _weight_blocks(W8).reshape(JB, 128, KP, 2, 128)  # [j, p, kk, slot, ai]
    rev = blk[..., ::-1]  # reverse ai
    out = np.empty((JB, 128, KP, 128, 2), dtype=W8.dtype)
    out[..., 0] = rev[:, :, :, 0, :]
    out[..., 1] = rev[:, :, :, 1, :]
    return np.ascontiguousarray(out.reshape(JB, 128, KB * 128))


def _make_in_maps(query, keys, mask, W1, W2, v):
    query = np.asarray(query, dtype=np.float32)
    keys = np.asarray(keys, dtype=np.float32)
    mask = np.asarray(mask)
    W1 = np.asarray(W1, dtype=np.float32)
    W2 = np.asarray(W2, dtype=np.float32)
    v = np.asarray(v, dtype=np.float32)

    import ml_dtypes

    f8np = ml_dtypes.float8_e4m3  # TRN float8e4 (max normal 240)
    w2t = _weight_blocks_swil(W2.astype(f8np))
    w1t = _weight_blocks(W1).astype(ml_dtypes.bfloat16)
    vcol = np.ascontiguousarray(v.reshape(JB, 128).T)  # [p, j]
    onesz = np.zeros((128, BL, BL), dtype=np.float32)
    for b in range(BL):
        onesz[:, b, b] = 1.0
    onesz = np.ascontiguousarray(onesz.reshape(128, BL * BL))

    in_maps = []
    for c in range(NCORES):
        sl = slice(c * BL, (c + 1) * BL)
        keysT_c = np.ascontiguousarray(keys[sl].transpose(2, 0, 1))  # (D, BL, S)
        khi = keysT_c.astype(f8np)
        klo = (keysT_c - khi.astype(np.float32)).astype(f8np)
        keysT8_c = np.stack([khi, klo], axis=0)  # (2, D, BL, S)
        qT_c = np.ascontiguousarray(
            query[sl].T.reshape(KB, 128, BL).transpose(1, 0, 2)
        )  # (128, KB, BL)
        qTb_c = qT_c.astype(ml_dtypes.bfloat16)
        maskadd_c = np.where(mask[sl], 0.0, -MASK_NEG).astype(np.float32)
        in_maps.append(
            {
                "keysT8": keysT8_c,
                "w2t": w2t,
                "w1t": w1t,
                "qT": qT_c,
                "qTb": qTb_c,
                "vcol": vcol,
                "onesz": onesz,
                "maskadd": maskadd_c,
            }
        )
    return in_maps


def kernel(query, keys, mask, W1, W2, v):
    from concourse.bass_utils import run_bass_kernel_spmd

    nc = _get_nc()
    in_maps = _make_in_maps(query, keys, mask, W1, W2, v)
    res = run_bass_kernel_spmd(nc, in_maps, core_ids=list(range(NCORES)))
    _cache["last_results"] = res
    out = np.concatenate([res.results[i]["out"] for i in range(NCORES)], axis=0)
    return out.astype(np.float32)



# revision 18
# speedup vs baseline: 1.8078x; 1.6419x over previous
"""Additive (Bahdanau) attention kernel for one TRN2 chip (8 NeuronCores).

Computes, for query (B,D), keys (B,S,D), mask (B,S), W1 (A,D), W2 (A,D), v (A,):
    scores[b,s] = v . tanh(W1 @ query[b] + W2 @ keys[b,s])
    out = softmax(scores - 1e30 * ~mask, axis=-1)

Sharding: data-parallel over batch B across the 8 cores (4 batches/core);
W1/W2/v replicated. No collectives needed; per-core outputs are concatenated
on the host.

Key optimizations over a straightforward fp32r formulation:
  - mask compaction (exact): masked-out key columns contribute exactly 0 to
    the softmax, so the host gathers only the active columns per batch
    (~half of S), pads to a multiple of ST, and scatters the output back.
  - fp8 e4m3 DoubleRow matmuls: keys are split host-side as
    k = e4m3(k) + e4m3(k - e4m3(k)) (hi/lo); each DoubleRow instruction
    contracts a 256-dim k-pair at 2 fp8 elements/cycle. W2 is single e4m3.
    Measured end-to-end rel err 0.0163 < 2e-2 gate.

Per-core device kernel:
  - w1q[a,b]    = W1 @ q_b              (tiny bf16 matmul, a on partitions)
  - per (s-tile of 384, b), per a-block j of 128:
        psum[a,s] += sum over 4 k-pairs of DoubleRow(W2_pair, khi_pair)
                     + DoubleRow(W2_pair, klo_pair)   (shared stationary)
        comb = tanh(psum + w1q_j[:,b])         (ScalarE, per-partition bias)
        acc  = (comb * v_j) + acc              (VectorE fused mul-add)
    a one-hot ones matmul per (s-tile, b) partition-reduces acc into row b
    of a shared [4, ST] psum tile
  - scores tile = sc_psum + maskadd; exp + partial row-sum per tile are
    emitted as soon as the tile's scores land (hidden under the main loop);
    the tail is only reciprocal + scale + output DMA.

Weights are stored per a-block (contiguous DRAM blocks, one SBUF tile each)
so Tile's per-tile dependency tracking lets the j-th matmul group start as
soon as its own block has landed. Keys stream on the sync DMA queue, weights
on the scalar queue; a short burst of junk matmuls warms the PE HAM clock
gate during the initial DMA wait.
"""

import numpy as np

B, S, D, A = 32, 2048, 1024, 1024
NCORES = 8
BL = B // NCORES  # 4 batches per core
ST = 384          # s-tile width
KB = D // 128     # 8 contraction blocks
KP = KB // 2      # 4 DoubleRow contraction pairs
JB = A // 128     # 8 attn-dim blocks
MASK_NEG = 1e30

_cache = {}


def _build_nc(nst):
    from contextlib import ExitStack

    import concourse.tile as tile
    from concourse import bacc, mybir

    f32 = mybir.dt.float32
    f32r = mybir.dt.float32r
    bf16 = mybir.dt.bfloat16
    f8 = mybir.dt.float8e4
    DR = mybir.MatmulPerfMode.DoubleRow
    Mult = mybir.AluOpType.mult
    Add = mybir.AluOpType.add
    Tanh = mybir.ActivationFunctionType.Tanh
    Exp = mybir.ActivationFunctionType.Exp

    sp = nst * ST  # padded active-column count

    nc = bacc.Bacc(
        "TRN2",
        target_bir_lowering=False,
        debug=False,
        enable_asserts=False,
        num_devices=NCORES,
    )

    # hi/lo e4m3 split of compacted keys: keysT8[i, d, b, s], i=0 hi, i=1 lo
    keysT8 = nc.dram_tensor("keysT8", [2, D, BL, sp], f8, kind="ExternalInput").ap()
    # per-a-block weight blocks: [j, p, k*128+ai] = W[j*128+ai, k*128+p]
    w2t = nc.dram_tensor("w2t", [JB, 128, KB * 128], f8, kind="ExternalInput").ap()
    w1t = nc.dram_tensor("w1t", [JB, 128, KB * 128], bf16, kind="ExternalInput").ap()
    qT = nc.dram_tensor("qT", [128, KB, BL], f32r, kind="ExternalInput").ap()
    qTb = nc.dram_tensor("qTb", [128, KB, BL], bf16, kind="ExternalInput").ap()
    # vcol[p, j] = v[j*128+p] — per-partition scalar for the DVE multiply
    vcol = nc.dram_tensor("vcol", [128, JB], f32, kind="ExternalInput").ap()
    # onesz[p, b, c] = 1 if b == c else 0 — one-hot ones column per batch so
    # each batch's partition-reduce lands in its own psum row
    onesz = nc.dram_tensor("onesz", [128, BL * BL], f32r, kind="ExternalInput").ap()
    maskadd = nc.dram_tensor("maskadd", [BL, sp], f32, kind="ExternalInput").ap()
    out = nc.dram_tensor("out", [BL, sp], f32, kind="ExternalOutput").ap()

    keysT_r = keysT8.rearrange("i (k p) b s -> p i k b s", p=128)

    with tile.TileContext(nc) as tc, ExitStack() as ctx:
        singles = ctx.enter_context(tc.tile_pool(name="singles", bufs=1))
        keysp = ctx.enter_context(tc.tile_pool(name="keys", bufs=2))
        combp = ctx.enter_context(tc.tile_pool(name="comb", bufs=3))
        accp = ctx.enter_context(tc.tile_pool(name="acc", bufs=3))
        accrp = ctx.enter_context(tc.tile_pool(name="accr", bufs=2))
        psmain = ctx.enter_context(tc.tile_pool(name="psmain", bufs=2, space="PSUM"))
        psvdot = ctx.enter_context(tc.tile_pool(name="psvdot", bufs=2, space="PSUM"))
        psw1q = ctx.enter_context(tc.tile_pool(name="psw1q", bufs=2, space="PSUM"))

        # --- staged input DMAs ---------------------------------------------
        # sync HWDGE queue (fast): q for warmup, W2 j=0, first keys tile,
        # then W2 j=1..7 ahead of the keys stream
        q_sb = singles.tile([128, KB, BL], f32r)
        nc.sync.dma_start(q_sb[:], qT)
        w2_sbj = [singles.tile([128, KB * 128], f8, name=f"w2_sb{j}") for j in range(JB)]
        nc.sync.dma_start(w2_sbj[0][:], w2t[0])
        kt0 = keysp.tile([128, 2, KB, ST], f8)
        nc.sync.dma_start(kt0[:], keysT_r[:, :, :, 0, 0:ST])
        for j in range(1, JB):
            nc.sync.dma_start(w2_sbj[j][:], w2t[j])

        # scalar HWDGE queue (slower, starts later): everything the tanh-bias
        # path needs — W1 in bf16 halves its bytes so w1q_j7 beats its deadline
        v_sb = singles.tile([128, JB], f32)
        nc.scalar.dma_start(v_sb[:], vcol)
        o_one = singles.tile([128, BL * BL], f32r)
        nc.scalar.dma_start(o_one[:], onesz)
        qb_sb = singles.tile([128, KB, BL], bf16)
        nc.scalar.dma_start(qb_sb[:], qTb)
        w1_sbj = [singles.tile([128, KB * 128], bf16, name=f"w1_sb{j}") for j in range(JB)]
        for j in range(JB):
            nc.scalar.dma_start(w1_sbj[j][:], w1t[j])
        ma_sb = singles.tile([BL, sp], f32)
        nc.scalar.dma_start(ma_sb[:], maskadd)

        scores = singles.tile([BL, sp], f32)
        e_sb = singles.tile([BL, sp], f32)
        sums = singles.tile([BL, nst], f32)
        w1qj = [singles.tile([128, BL], f32, name=f"w1q{j}") for j in range(JB)]

        # preload the exp_and_others ACT table set (covers Tanh+Exp+Copy)
        # during the initial DMA wait instead of stalling the first tanh
        dummy_act = singles.tile([128, 1], f32)
        nc.scalar.activation(dummy_act[:], v_sb[:, 0:1], Tanh)

        # HAM warmup: junk matmuls on the first-arriving input keep the PE
        # busy through the clock-gate window while the real data streams in
        warm_ps = psw1q.tile([BL, KB * BL], f32)
        q_flat = q_sb[:].rearrange("p k b -> p (k b)")
        NWARM = 160
        for w in range(NWARM):
            nc.tensor.matmul(
                warm_ps[:],
                lhsT=q_sb[:, w % KB, :],
                rhs=q_flat,
                start=(w == 0),
                stop=(w == NWARM - 1),
            )

        def emit_w1q(j):
            # w1q_j[a, b] = sum_d W1[a, d] q[b, d], a on partitions
            wq_ps = psw1q.tile([128, BL], f32)
            for k in range(KB):
                nc.tensor.matmul(
                    wq_ps[:],
                    lhsT=w1_sbj[j][:, k * 128 : (k + 1) * 128],
                    rhs=qb_sb[:, k, :],
                    start=(k == 0),
                    stop=(k == KB - 1),
                )
            nc.scalar.copy(w1qj[j][:], wq_ps[:])

        # --- main loop ------------------------------------------------------
        # the one-hot partition-reduce matmul for (st, b) is emitted two main
        # j-groups later so the in-order PE never waits on the tanh+DVE chain
        groups_done = 0
        ones_queue = []  # (sc_ps, accr, b, st, group_when_ready)

        def flush_ones(min_age):
            while ones_queue and groups_done - ones_queue[0][4] >= min_age:
                sc_ps_q, accr_q, b_q, st_q, _ = ones_queue.pop(0)
                nc.tensor.matmul(
                    sc_ps_q[:],
                    lhsT=o_one[:, b_q * BL : (b_q + 1) * BL],
                    rhs=accr_q[:],
                    start=(b_q == 0),
                    stop=(b_q == BL - 1),
                )
                if b_q == BL - 1:
                    sl = slice(st_q * ST, (st_q + 1) * ST)
                    nc.vector.tensor_add(scores[:, sl], sc_ps_q[:, :], ma_sb[:, sl])
                    # masked softmax, pipelined: exp + partial row-sum for
                    # this tile run under the remaining main loop
                    nc.scalar.activation(e_sb[:, sl], scores[:, sl], Exp)
                    nc.vector.reduce_sum(
                        sums[:, st_q : st_q + 1],
                        e_sb[:, sl],
                        axis=mybir.AxisListType.X,
                    )

        sc_tiles = [psvdot.tile([BL, ST], f32, name=f"sc_ps{st}", tag="sc_ps") for st in range(nst)]
        for st in range(nst):
            sc_ps = sc_tiles[st]
            for b in range(BL):
                if st == 0 and b == 0:
                    kt = kt0
                else:
                    kt = keysp.tile([128, 2, KB, ST], f8)
                    nc.sync.dma_start(
                        kt[:], keysT_r[:, :, :, b, st * ST : (st + 1) * ST]
                    )
                acc = accp.tile([128, ST], f32)
                accr = accrp.tile([128, ST], f32r)
                for j in range(JB):
                    if st == 0 and b == 0:
                        # just-in-time w1q: emitted into the PE stream right
                        # before the main group that its tanh will need
                        emit_w1q(j)
                    ps = psmain.tile([128, ST], f32)
                    for kk in range(KP):
                        # DoubleRow pair: 256-dim contraction per instr; the
                        # hi and lo passes share the stationary weight pair
                        w3 = w2_sbj[j][:, kk * 256 : (kk + 1) * 256].rearrange(
                            "p (two m) -> p two m", two=2
                        )
                        nc.tensor.matmul(
                            ps[:],
                            lhsT=w3,
                            rhs=kt[:, 0, 2 * kk : 2 * kk + 2, :],
                            start=(kk == 0),
                            stop=False,
                            perf_mode=DR,
                        )
                        nc.tensor.matmul(
                            ps[:],
                            lhsT=w3,
                            rhs=kt[:, 1, 2 * kk : 2 * kk + 2, :],
                            start=False,
                            stop=(kk == KP - 1),
                            perf_mode=DR,
                        )
                    groups_done += 1
                    flush_ones(2)
                    comb = combp.tile([128, ST], f32)
                    nc.scalar.activation(
                        comb[:], ps[:], Tanh, bias=w1qj[j][:, b : b + 1]
                    )
                    # acc = (comb * v_j) + acc fused on VectorE
                    if j == 0:
                        nc.vector.tensor_scalar_mul(acc[:], comb[:], v_sb[:, 0:1])
                    elif j == JB - 1:
                        nc.vector.scalar_tensor_tensor(
                            accr[:], comb[:], v_sb[:, j : j + 1], acc[:], Mult, Add
                        )
                    else:
                        nc.vector.scalar_tensor_tensor(
                            acc[:], comb[:], v_sb[:, j : j + 1], acc[:], Mult, Add
                        )
                ones_queue.append((sc_ps, accr, b, st, groups_done))
        flush_ones(0)

        # --- softmax tail: combine partial sums, scale, write out -----------
        # scores are bounded by sum|v| (~27), so exp cannot overflow f32 and
        # the max subtraction is unnecessary; masked/padded entries are
        # exp(-1e30)=0. exp + partial sums already ran per tile above.
        sm = singles.tile([BL, 1], f32)
        nc.vector.reduce_sum(sm[:], sums[:, 0:nst], axis=mybir.AxisListType.X)
        rs = singles.tile([BL, 1], f32)
        nc.vector.reciprocal(rs[:], sm[:])
        o_sb = singles.tile([BL, sp], f32)
        H = sp // 2
        for h in range(2):
            nc.vector.tensor_scalar_mul(
                o_sb[:, h * H : (h + 1) * H], e_sb[:, h * H : (h + 1) * H], rs[:, 0:1]
            )
            nc.sync.dma_start(out[:, h * H : (h + 1) * H], o_sb[:, h * H : (h + 1) * H])

    nc.compile()
    return nc


def _get_nc(nst):
    key = ("nc", nst)
    if key not in _cache:
        _cache[key] = _build_nc(nst)
    return _cache[key]


def _weight_blocks(W):
    # [j, p, k*128+ai] = W[j*128+ai, k*128+p]
    return np.ascontiguousarray(
        W.reshape(JB, 128, KB, 128).transpose(0, 3, 2, 1).reshape(JB, 128, KB * 128)
    )


def _make_in_maps(query, keys, mask, W1, W2, v, sp):
    query = np.asarray(query, dtype=np.float32)
    keys = np.asarray(keys, dtype=np.float32)
    mask = np.asarray(mask)
    W1 = np.asarray(W1, dtype=np.float32)
    W2 = np.asarray(W2, dtype=np.float32)
    v = np.asarray(v, dtype=np.float32)

    import ml_dtypes

    f8np = ml_dtypes.float8_e4m3  # TRN float8e4 (max normal 240)
    w2t = _weight_blocks(W2).astype(f8np)
    w1t = _weight_blocks(W1).astype(ml_dtypes.bfloat16)
    vcol = np.ascontiguousarray(v.reshape(JB, 128).T)  # [p, j]
    onesz = np.zeros((128, BL, BL), dtype=np.float32)
    for b in range(BL):
        onesz[:, b, b] = 1.0
    onesz = np.ascontiguousarray(onesz.reshape(128, BL * BL))

    in_maps = []
    idx_all = []
    for c in range(NCORES):
        # mask compaction: gather only active key columns, pad to sp
        kc = np.zeros((BL, sp, D), dtype=np.float32)
        maskadd_c = np.full((BL, sp), -MASK_NEG, dtype=np.float32)
        idx_core = []
        for bl in range(BL):
            gb = c * BL + bl
            idx = np.flatnonzero(mask[gb])
            kc[bl, : len(idx)] = keys[gb][idx]
            maskadd_c[bl, : len(idx)] = 0.0
            idx_core.append(idx)
        idx_all.append(idx_core)
        keysT_c = np.ascontiguousarray(kc.transpose(2, 0, 1))  # (D, BL, sp)
        khi = keysT_c.astype(f8np)
        klo = (keysT_c - khi.astype(np.float32)).astype(f8np)
        keysT8_c = np.stack([khi, klo], axis=0)  # (2, D, BL, sp)
        qT_c = np.ascontiguousarray(
            query[c * BL : (c + 1) * BL].T.reshape(KB, 128, BL).transpose(1, 0, 2)
        )  # (128, KB, BL)
        qTb_c = qT_c.astype(ml_dtypes.bfloat16)
        in_maps.append(
            {
                "keysT8": keysT8_c,
                "w2t": w2t,
                "w1t": w1t,
                "qT": qT_c,
                "qTb": qTb_c,
                "vcol": vcol,
                "onesz": onesz,
                "maskadd": maskadd_c,
            }
        )
    return in_maps, idx_all


def kernel(query, keys, mask, W1, W2, v):
    from concourse.bass_utils import run_bass_kernel_spmd

    mask_np = np.asarray(mask)
    n_max = int(mask_np.sum(axis=1).max())
    nst = max(1, -(-n_max // ST))  # ceil
    sp = nst * ST

    nc = _get_nc(nst)
    in_maps, idx_all = _make_in_maps(query, keys, mask_np, W1, W2, v, sp)
    res = run_bass_kernel_spmd(nc, in_maps, core_ids=list(range(NCORES)))
    _cache["last_results"] = res

    out = np.zeros((B, S), dtype=np.float32)
    for c in range(NCORES):
        oc = res.results[c]["out"]
        for bl in range(BL):
            idx = idx_all[c][bl]
            out[c * BL + bl, idx] = oc[bl, : len(idx)]
    return out


# revision 19
# speedup vs baseline: 1.9546x; 1.0812x over previous
"""Additive (Bahdanau) attention kernel for one TRN2 chip (8 NeuronCores).

Computes, for query (B,D), keys (B,S,D), mask (B,S), W1 (A,D), W2 (A,D), v (A,):
    scores[b,s] = v . tanh(W1 @ query[b] + W2 @ keys[b,s])
    out = softmax(scores - 1e30 * ~mask, axis=-1)

Sharding: data-parallel over batch B across the 8 cores (4 batches/core);
W1/W2/v replicated. No collectives needed; per-core outputs are concatenated
on the host.

Key optimizations over a straightforward fp32r formulation:
  - mask compaction (exact): masked-out key columns contribute exactly 0 to
    the softmax, so the host gathers only the active columns per batch
    (~half of S), pads to a multiple of ST, and scatters the output back.
  - fp8 e4m3 DoubleRow matmuls: keys are split host-side as
    k = e4m3(k) + e4m3(k - e4m3(k)) (hi/lo); each DoubleRow instruction
    contracts a 256-dim k-pair at 2 fp8 elements/cycle. W2 is single e4m3.
    Measured end-to-end rel err 0.0163 < 2e-2 gate.

Per-core device kernel:
  - w1q[a,b]    = W1 @ q_b              (tiny bf16 matmul, a on partitions)
  - per (s-tile of 384, b), per a-block j of 128:
        psum[a,s] += sum over 4 k-pairs of DoubleRow(W2_pair, khi_pair)
                     + DoubleRow(W2_pair, klo_pair)   (shared stationary)
        comb = tanh(psum + w1q_j[:,b])         (ScalarE, per-partition bias)
        acc  = (comb * v_j) + acc              (VectorE fused mul-add)
    a one-hot ones matmul per (s-tile, b) partition-reduces acc into row b
    of a shared [4, ST] psum tile
  - scores tile = sc_psum + maskadd; exp + partial row-sum per tile are
    emitted as soon as the tile's scores land (hidden under the main loop);
    the tail is only reciprocal + scale + output DMA.

Weights are stored per a-block (contiguous DRAM blocks, one SBUF tile each)
so Tile's per-tile dependency tracking lets the j-th matmul group start as
soon as its own block has landed. Keys stream on the sync DMA queue, weights
on the scalar queue; a short burst of junk matmuls warms the PE HAM clock
gate during the initial DMA wait.
"""

import numpy as np

B, S, D, A = 32, 2048, 1024, 1024
NCORES = 8
BL = B // NCORES  # 4 batches per core
ST = 384          # s-tile width
KB = D // 128     # 8 contraction blocks
KP = KB // 2      # 4 DoubleRow contraction pairs
JB = A // 128     # 8 attn-dim blocks
MASK_NEG = 1e30

_cache = {}


def _build_nc(nst):
    from contextlib import ExitStack

    import concourse.tile as tile
    from concourse import bacc, mybir

    f32 = mybir.dt.float32
    f32r = mybir.dt.float32r
    bf16 = mybir.dt.bfloat16
    f8 = mybir.dt.float8e4
    DR = mybir.MatmulPerfMode.DoubleRow
    Mult = mybir.AluOpType.mult
    Add = mybir.AluOpType.add
    Tanh = mybir.ActivationFunctionType.Tanh
    Exp = mybir.ActivationFunctionType.Exp

    sp = nst * ST  # padded active-column count

    nc = bacc.Bacc(
        "TRN2",
        target_bir_lowering=False,
        debug=False,
        enable_asserts=False,
        num_devices=NCORES,
    )

    # hi/lo e4m3 split of compacted keys: keysT8[i, d, b, s], i=0 hi, i=1 lo
    keysT8 = nc.dram_tensor("keysT8", [2, D, BL, sp], f8, kind="ExternalInput").ap()
    # per-a-block weight blocks: [j, p, k*128+ai] = W[j*128+ai, k*128+p]
    w2t = nc.dram_tensor("w2t", [JB, 128, KB * 128], f8, kind="ExternalInput").ap()
    w1t = nc.dram_tensor("w1t", [JB, 128, KB * 128], bf16, kind="ExternalInput").ap()
    qT = nc.dram_tensor("qT", [128, KB, BL], f32r, kind="ExternalInput").ap()
    qTb = nc.dram_tensor("qTb", [128, KB, BL], bf16, kind="ExternalInput").ap()
    # vcol[p, j] = v[j*128+p] — per-partition scalar for the DVE multiply
    vcol = nc.dram_tensor("vcol", [128, JB], f32, kind="ExternalInput").ap()
    # onesz[p, b, c] = 1 if b == c else 0 — one-hot ones column per batch so
    # each batch's partition-reduce lands in its own psum row
    onesz = nc.dram_tensor("onesz", [128, BL * BL], f32r, kind="ExternalInput").ap()
    maskadd = nc.dram_tensor("maskadd", [BL, sp], f32, kind="ExternalInput").ap()
    out = nc.dram_tensor("out", [BL, sp], f32, kind="ExternalOutput").ap()

    keysT_r = keysT8.rearrange("i (k p) b s -> p i k b s", p=128)

    with tile.TileContext(nc) as tc, ExitStack() as ctx:
        singles = ctx.enter_context(tc.tile_pool(name="singles", bufs=1))
        keysp = ctx.enter_context(tc.tile_pool(name="keys", bufs=2))
        combp = ctx.enter_context(tc.tile_pool(name="comb", bufs=3))
        accp = ctx.enter_context(tc.tile_pool(name="acc", bufs=3))
        accrp = ctx.enter_context(tc.tile_pool(name="accr", bufs=2))
        psmain = ctx.enter_context(tc.tile_pool(name="psmain", bufs=2, space="PSUM"))
        psvdot = ctx.enter_context(tc.tile_pool(name="psvdot", bufs=2, space="PSUM"))
        psw1q = ctx.enter_context(tc.tile_pool(name="psw1q", bufs=2, space="PSUM"))

        # --- staged input DMAs ---------------------------------------------
        # sync HWDGE queue (fast): q for warmup, W2 j=0, first keys tile,
        # then W2 j=1..7 ahead of the keys stream
        q_sb = singles.tile([128, KB, BL], f32r)
        nc.sync.dma_start(q_sb[:], qT)
        w2_sbj = [singles.tile([128, KB * 128], f8, name=f"w2_sb{j}") for j in range(JB)]
        nc.sync.dma_start(w2_sbj[0][:], w2t[0])
        kt0 = keysp.tile([128, 2, KB, ST], f8)
        nc.sync.dma_start(kt0[:], keysT_r[:, :, :, 0, 0:ST])
        for j in range(1, JB):
            nc.sync.dma_start(w2_sbj[j][:], w2t[j])

        # scalar HWDGE queue (slower, starts later): everything the tanh-bias
        # path needs — W1 in bf16 halves its bytes so w1q_j7 beats its deadline
        v_sb = singles.tile([128, JB], f32)
        nc.scalar.dma_start(v_sb[:], vcol)
        o_one = singles.tile([128, BL * BL], f32r)
        nc.scalar.dma_start(o_one[:], onesz)
        qb_sb = singles.tile([128, KB, BL], bf16)
        nc.scalar.dma_start(qb_sb[:], qTb)
        w1_sbj = [singles.tile([128, KB * 128], bf16, name=f"w1_sb{j}") for j in range(JB)]
        for j in range(JB):
            nc.scalar.dma_start(w1_sbj[j][:], w1t[j])
        ma_sb = singles.tile([BL, sp], f32)
        nc.scalar.dma_start(ma_sb[:], maskadd)

        scores = singles.tile([BL, sp], f32)
        e_sb = singles.tile([BL, sp], f32)
        sums = singles.tile([BL, nst], f32)
        w1qj = [singles.tile([128, BL], f32, name=f"w1q{j}") for j in range(JB)]

        # preload the exp_and_others ACT table set (covers Tanh+Exp+Copy)
        # during the initial DMA wait instead of stalling the first tanh
        dummy_act = singles.tile([128, 1], f32)
        nc.scalar.activation(dummy_act[:], v_sb[:, 0:1], Tanh)

        # HAM warmup: junk matmuls on the first-arriving input keep the PE
        # busy through the clock-gate window while the real data streams in
        warm_ps = psw1q.tile([BL, KB * BL], f32)
        q_flat = q_sb[:].rearrange("p k b -> p (k b)")
        NWARM = 160
        for w in range(NWARM):
            nc.tensor.matmul(
                warm_ps[:],
                lhsT=q_sb[:, w % KB, :],
                rhs=q_flat,
                start=(w == 0),
                stop=(w == NWARM - 1),
            )

        def emit_w1q(j):
            # w1q_j[a, b] = sum_d W1[a, d] q[b, d], a on partitions
            wq_ps = psw1q.tile([128, BL], f32)
            for k in range(KB):
                nc.tensor.matmul(
                    wq_ps[:],
                    lhsT=w1_sbj[j][:, k * 128 : (k + 1) * 128],
                    rhs=qb_sb[:, k, :],
                    start=(k == 0),
                    stop=(k == KB - 1),
                )
            nc.scalar.copy(w1qj[j][:], wq_ps[:])

        # --- main loop ------------------------------------------------------
        # the one-hot partition-reduce matmul for (st, b) is emitted two main
        # j-groups later so the in-order PE never waits on the tanh+DVE chain
        groups_done = 0
        ones_queue = []  # (sc_ps, accr, b, st, group_when_ready)

        def flush_ones(min_age):
            while ones_queue and groups_done - ones_queue[0][4] >= min_age:
                sc_ps_q, accr_q, b_q, st_q, _ = ones_queue.pop(0)
                nc.tensor.matmul(
                    sc_ps_q[:],
                    lhsT=o_one[:, b_q * BL : (b_q + 1) * BL],
                    rhs=accr_q[:],
                    start=(b_q == 0),
                    stop=(b_q == BL - 1),
                )
                if b_q == BL - 1:
                    sl = slice(st_q * ST, (st_q + 1) * ST)
                    nc.vector.tensor_add(scores[:, sl], sc_ps_q[:, :], ma_sb[:, sl])
                    # masked softmax, pipelined: exp + partial row-sum for
                    # this tile run under the remaining main loop
                    nc.scalar.activation(e_sb[:, sl], scores[:, sl], Exp)
                    nc.vector.reduce_sum(
                        sums[:, st_q : st_q + 1],
                        e_sb[:, sl],
                        axis=mybir.AxisListType.X,
                    )

        sc_tiles = [psvdot.tile([BL, ST], f32, name=f"sc_ps{st}", tag="sc_ps") for st in range(nst)]
        for st in range(nst):
            sc_ps = sc_tiles[st]
            for b in range(BL):
                if st == 0 and b == 0:
                    kt = kt0
                else:
                    kt = keysp.tile([128, 2, KB, ST], f8)
                    nc.sync.dma_start(
                        kt[:], keysT_r[:, :, :, b, st * ST : (st + 1) * ST]
                    )
                acc = accp.tile([128, ST], f32)
                accr = accrp.tile([128, ST], f32r)
                for j in range(JB):
                    if st == 0 and b == 0:
                        # just-in-time w1q: emitted into the PE stream right
                        # before the main group that its tanh will need
                        emit_w1q(j)
                    ps = psmain.tile([128, ST], f32)
                    for kk in range(KP):
                        # DoubleRow pair: 256-dim contraction per instr; the
                        # hi and lo passes share the stationary weight pair.
                        # The last pair skips its lo correction: the keys
                        # quantization error of 2/8 k-blocks is negligible
                        # (simulated rel err 0.0162 vs 0.0163 fully corrected)
                        # and it saves 1/8 of the main matmul instructions.
                        w3 = w2_sbj[j][:, kk * 256 : (kk + 1) * 256].rearrange(
                            "p (two m) -> p two m", two=2
                        )
                        last = kk == KP - 1
                        nc.tensor.matmul(
                            ps[:],
                            lhsT=w3,
                            rhs=kt[:, 0, 2 * kk : 2 * kk + 2, :],
                            start=(kk == 0),
                            stop=last,
                            perf_mode=DR,
                        )
                        if not last:
                            nc.tensor.matmul(
                                ps[:],
                                lhsT=w3,
                                rhs=kt[:, 1, 2 * kk : 2 * kk + 2, :],
                                start=False,
                                stop=False,
                                perf_mode=DR,
                            )
                    groups_done += 1
                    flush_ones(2)
                    comb = combp.tile([128, ST], f32)
                    nc.scalar.activation(
                        comb[:], ps[:], Tanh, bias=w1qj[j][:, b : b + 1]
                    )
                    # acc = (comb * v_j) + acc fused on VectorE
                    if j == 0:
                        nc.vector.tensor_scalar_mul(acc[:], comb[:], v_sb[:, 0:1])
                    elif j == JB - 1:
                        nc.vector.scalar_tensor_tensor(
                            accr[:], comb[:], v_sb[:, j : j + 1], acc[:], Mult, Add
                        )
                    else:
                        nc.vector.scalar_tensor_tensor(
                            acc[:], comb[:], v_sb[:, j : j + 1], acc[:], Mult, Add
                        )
                ones_queue.append((sc_ps, accr, b, st, groups_done))
        flush_ones(0)

        # --- softmax tail: combine partial sums, scale, write out -----------
        # scores are bounded by sum|v| (~27), so exp cannot overflow f32 and
        # the max subtraction is unnecessary; masked/padded entries are
        # exp(-1e30)=0. exp + partial sums already ran per tile above.
        sm = singles.tile([BL, 1], f32)
        nc.vector.reduce_sum(sm[:], sums[:, 0:nst], axis=mybir.AxisListType.X)
        rs = singles.tile([BL, 1], f32)
        nc.vector.reciprocal(rs[:], sm[:])
        o_sb = singles.tile([BL, sp], f32)
        H = sp // 2
        for h in range(2):
            nc.vector.tensor_scalar_mul(
                o_sb[:, h * H : (h + 1) * H], e_sb[:, h * H : (h + 1) * H], rs[:, 0:1]
            )
            nc.sync.dma_start(out[:, h * H : (h + 1) * H], o_sb[:, h * H : (h + 1) * H])

    nc.compile()
    return nc


def _get_nc(nst):
    key = ("nc", nst)
    if key not in _cache:
        _cache[key] = _build_nc(nst)
    return _cache[key]


def _weight_blocks(W):
    # [j, p, k*128+ai] = W[j*128+ai, k*128+p]
    return np.ascontiguousarray(
        W.reshape(JB, 128, KB, 128).transpose(0, 3, 2, 1).reshape(JB, 128, KB * 128)
    )


def _make_in_maps(query, keys, mask, W1, W2, v, sp):
    query = np.asarray(query, dtype=np.float32)
    keys = np.asarray(keys, dtype=np.float32)
    mask = np.asarray(mask)
    W1 = np.asarray(W1, dtype=np.float32)
    W2 = np.asarray(W2, dtype=np.float32)
    v = np.asarray(v, dtype=np.float32)

    import ml_dtypes

    f8np = ml_dtypes.float8_e4m3  # TRN float8e4 (max normal 240)
    w2t = _weight_blocks(W2).astype(f8np)
    w1t = _weight_blocks(W1).astype(ml_dtypes.bfloat16)
    vcol = np.ascontiguousarray(v.reshape(JB, 128).T)  # [p, j]
    onesz = np.zeros((128, BL, BL), dtype=np.float32)
    for b in range(BL):
        onesz[:, b, b] = 1.0
    onesz = np.ascontiguousarray(onesz.reshape(128, BL * BL))

    in_maps = []
    idx_all = []
    for c in range(NCORES):
        # mask compaction: gather only active key columns, pad to sp
        kc = np.zeros((BL, sp, D), dtype=np.float32)
        maskadd_c = np.full((BL, sp), -MASK_NEG, dtype=np.float32)
        idx_core = []
        for bl in range(BL):
            gb = c * BL + bl
            idx = np.flatnonzero(mask[gb])
            kc[bl, : len(idx)] = keys[gb][idx]
            maskadd_c[bl, : len(idx)] = 0.0
            idx_core.append(idx)
        idx_all.append(idx_core)
        keysT_c = np.ascontiguousarray(kc.transpose(2, 0, 1))  # (D, BL, sp)
        khi = keysT_c.astype(f8np)
        klo = (keysT_c - khi.astype(np.float32)).astype(f8np)
        keysT8_c = np.stack([khi, klo], axis=0)  # (2, D, BL, sp)
        qT_c = np.ascontiguousarray(
            query[c * BL : (c + 1) * BL].T.reshape(KB, 128, BL).transpose(1, 0, 2)
        )  # (128, KB, BL)
        qTb_c = qT_c.astype(ml_dtypes.bfloat16)
        in_maps.append(
            {
                "keysT8": keysT8_c,
                "w2t": w2t,
                "w1t": w1t,
                "qT": qT_c,
                "qTb": qTb_c,
                "vcol": vcol,
                "onesz": onesz,
                "maskadd": maskadd_c,
            }
        )
    return in_maps, idx_all


def kernel(query, keys, mask, W1, W2, v):
    from concourse.bass_utils import run_bass_kernel_spmd

    mask_np = np.asarray(mask)
    n_max = int(mask_np.sum(axis=1).max())
    nst = max(1, -(-n_max // ST))  # ceil
    sp = nst * ST

    nc = _get_nc(nst)
    in_maps, idx_all = _make_in_maps(query, keys, mask_np, W1, W2, v, sp)
    res = run_bass_kernel_spmd(nc, in_maps, core_ids=list(range(NCORES)))
    _cache["last_results"] = res

    out = np.zeros((B, S), dtype=np.float32)
    for c in range(NCORES):
        oc = res.results[c]["out"]
        for bl in range(BL):
            idx = idx_all[c][bl]
            out[c * BL + bl, idx] = oc[bl, : len(idx)]
    return out


# revision 20
# speedup vs baseline: 2.0340x; 1.0406x over previous
"""Additive (Bahdanau) attention kernel for one TRN2 chip (8 NeuronCores).

Computes, for query (B,D), keys (B,S,D), mask (B,S), W1 (A,D), W2 (A,D), v (A,):
    scores[b,s] = v . tanh(W1 @ query[b] + W2 @ keys[b,s])
    out = softmax(scores - 1e30 * ~mask, axis=-1)

Sharding: data-parallel over batch B across the 8 cores (4 batches/core);
W1/W2/v replicated. No collectives needed; per-core outputs are concatenated
on the host.

Key optimizations over a straightforward fp32r formulation:
  - mask compaction (exact): masked-out key columns contribute exactly 0 to
    the softmax, so the host gathers only the active columns per batch
    (~half of S), pads to a multiple of ST, and scatters the output back.
  - fp8 e4m3 DoubleRow matmuls: keys are split host-side as
    k = e4m3(k) + e4m3(k - e4m3(k)) (hi/lo); each DoubleRow instruction
    contracts a 256-dim k-pair at 2 fp8 elements/cycle. W2 is single e4m3.
    Measured end-to-end rel err 0.0163 < 2e-2 gate.

Per-core device kernel:
  - w1q[a,b]    = W1 @ q_b              (tiny bf16 matmul, a on partitions)
  - per (s-tile of 384, b), per a-block j of 128:
        psum[a,s] += sum over 4 k-pairs of DoubleRow(W2_pair, khi_pair)
                     + DoubleRow(W2_pair, klo_pair)   (shared stationary)
        comb = tanh(psum + w1q_j[:,b])         (ScalarE, per-partition bias)
        acc  = (comb * v_j) + acc              (VectorE fused mul-add)
    a one-hot ones matmul per (s-tile, b) partition-reduces acc into row b
    of a shared [4, ST] psum tile
  - scores tile = sc_psum + maskadd; exp + partial row-sum per tile are
    emitted as soon as the tile's scores land (hidden under the main loop);
    the tail is only reciprocal + scale + output DMA.

Weights are stored per a-block (contiguous DRAM blocks, one SBUF tile each)
so Tile's per-tile dependency tracking lets the j-th matmul group start as
soon as its own block has landed. Keys stream on the sync DMA queue, weights
on the scalar queue; a short burst of junk matmuls warms the PE HAM clock
gate during the initial DMA wait.
"""

import numpy as np

B, S, D, A = 32, 2048, 1024, 1024
NCORES = 8
BL = B // NCORES  # 4 batches per core
ST = 368          # s-tile width (multiple of 16 for DoubleRow APs)
KB = D // 128     # 8 contraction blocks
KP = KB // 2      # 4 DoubleRow contraction pairs
JB = A // 128     # 8 attn-dim blocks
MASK_NEG = 1e30

_cache = {}


def _build_nc(nst):
    from contextlib import ExitStack

    import concourse.tile as tile
    from concourse import bacc, mybir

    f32 = mybir.dt.float32
    f32r = mybir.dt.float32r
    bf16 = mybir.dt.bfloat16
    f8 = mybir.dt.float8e4
    DR = mybir.MatmulPerfMode.DoubleRow
    Mult = mybir.AluOpType.mult
    Add = mybir.AluOpType.add
    Tanh = mybir.ActivationFunctionType.Tanh
    Exp = mybir.ActivationFunctionType.Exp

    sp = nst * ST  # padded active-column count

    nc = bacc.Bacc(
        "TRN2",
        target_bir_lowering=False,
        debug=False,
        enable_asserts=False,
        num_devices=NCORES,
    )

    # hi/lo e4m3 split of compacted keys: keysT8[i, d, b, s], i=0 hi, i=1 lo
    keysT8 = nc.dram_tensor("keysT8", [2, D, BL, sp], f8, kind="ExternalInput").ap()
    # per-a-block weight blocks: [j, p, k*128+ai] = W[j*128+ai, k*128+p]
    w2t = nc.dram_tensor("w2t", [JB, 128, KB * 128], f8, kind="ExternalInput").ap()
    w1t = nc.dram_tensor("w1t", [JB, 128, KB * 128], bf16, kind="ExternalInput").ap()
    qT = nc.dram_tensor("qT", [128, KB, BL], f32r, kind="ExternalInput").ap()
    qTb = nc.dram_tensor("qTb", [128, KB, BL], bf16, kind="ExternalInput").ap()
    # vcol[p, j] = v[j*128+p] — per-partition scalar for the DVE multiply
    vcol = nc.dram_tensor("vcol", [128, JB], f32, kind="ExternalInput").ap()
    # onesz[p, b, c] = 1 if b == c else 0 — one-hot ones column per batch so
    # each batch's partition-reduce lands in its own psum row
    onesz = nc.dram_tensor("onesz", [128, BL * BL], f32r, kind="ExternalInput").ap()
    maskadd = nc.dram_tensor("maskadd", [BL, sp], f32, kind="ExternalInput").ap()
    out = nc.dram_tensor("out", [BL, sp], f32, kind="ExternalOutput").ap()

    keysT_r = keysT8.rearrange("i (k p) b s -> p i k b s", p=128)

    with tile.TileContext(nc) as tc, ExitStack() as ctx:
        singles = ctx.enter_context(tc.tile_pool(name="singles", bufs=1))
        keysp = ctx.enter_context(tc.tile_pool(name="keys", bufs=2))
        combp = ctx.enter_context(tc.tile_pool(name="comb", bufs=3))
        accp = ctx.enter_context(tc.tile_pool(name="acc", bufs=3))
        accrp = ctx.enter_context(tc.tile_pool(name="accr", bufs=2))
        psmain = ctx.enter_context(tc.tile_pool(name="psmain", bufs=2, space="PSUM"))
        psvdot = ctx.enter_context(tc.tile_pool(name="psvdot", bufs=2, space="PSUM"))
        psw1q = ctx.enter_context(tc.tile_pool(name="psw1q", bufs=2, space="PSUM"))

        # --- staged input DMAs ---------------------------------------------
        # sync HWDGE queue (fast): q for warmup, W2 j=0, first keys tile,
        # then W2 j=1..7 ahead of the keys stream
        q_sb = singles.tile([128, KB, BL], f32r)
        nc.sync.dma_start(q_sb[:], qT)
        w2_sbj = [singles.tile([128, KB * 128], f8, name=f"w2_sb{j}") for j in range(JB)]
        nc.sync.dma_start(w2_sbj[0][:], w2t[0])
        kt0 = keysp.tile([128, 2, KB, ST], f8)
        nc.sync.dma_start(kt0[:], keysT_r[:, :, :, 0, 0:ST])
        for j in range(1, JB):
            nc.sync.dma_start(w2_sbj[j][:], w2t[j])

        # scalar HWDGE queue (slower, starts later): everything the tanh-bias
        # path needs — W1 in bf16 halves its bytes so w1q_j7 beats its deadline
        v_sb = singles.tile([128, JB], f32)
        nc.scalar.dma_start(v_sb[:], vcol)
        o_one = singles.tile([128, BL * BL], f32r)
        nc.scalar.dma_start(o_one[:], onesz)
        qb_sb = singles.tile([128, KB, BL], bf16)
        nc.scalar.dma_start(qb_sb[:], qTb)
        w1_sbj = [singles.tile([128, KB * 128], bf16, name=f"w1_sb{j}") for j in range(JB)]
        for j in range(JB):
            nc.scalar.dma_start(w1_sbj[j][:], w1t[j])
        ma_sb = singles.tile([BL, sp], f32)
        nc.scalar.dma_start(ma_sb[:], maskadd)

        scores = singles.tile([BL, sp], f32)
        e_sb = singles.tile([BL, sp], f32)
        sums = singles.tile([BL, nst], f32)
        w1qj = [singles.tile([128, BL], f32, name=f"w1q{j}") for j in range(JB)]

        # preload the exp_and_others ACT table set (covers Tanh+Exp+Copy)
        # during the initial DMA wait instead of stalling the first tanh
        dummy_act = singles.tile([128, 1], f32)
        nc.scalar.activation(dummy_act[:], v_sb[:, 0:1], Tanh)

        # HAM warmup: junk matmuls on the first-arriving input keep the PE
        # busy through the clock-gate window while the real data streams in
        warm_ps = psw1q.tile([BL, KB * BL], f32)
        q_flat = q_sb[:].rearrange("p k b -> p (k b)")
        NWARM = 120
        for w in range(NWARM):
            nc.tensor.matmul(
                warm_ps[:],
                lhsT=q_sb[:, w % KB, :],
                rhs=q_flat,
                start=(w == 0),
                stop=(w == NWARM - 1),
            )

        def emit_w1q(j):
            # w1q_j[a, b] = sum_d W1[a, d] q[b, d], a on partitions
            wq_ps = psw1q.tile([128, BL], f32)
            for k in range(KB):
                nc.tensor.matmul(
                    wq_ps[:],
                    lhsT=w1_sbj[j][:, k * 128 : (k + 1) * 128],
                    rhs=qb_sb[:, k, :],
                    start=(k == 0),
                    stop=(k == KB - 1),
                )
            nc.scalar.copy(w1qj[j][:], wq_ps[:])

        # --- main loop ------------------------------------------------------
        # the one-hot partition-reduce matmul for (st, b) is emitted two main
        # j-groups later so the in-order PE never waits on the tanh+DVE chain
        groups_done = 0
        ones_queue = []  # (sc_ps, accr, b, st, group_when_ready)

        def flush_ones(min_age):
            while ones_queue and groups_done - ones_queue[0][4] >= min_age:
                sc_ps_q, accr_q, b_q, st_q, _ = ones_queue.pop(0)
                nc.tensor.matmul(
                    sc_ps_q[:],
                    lhsT=o_one[:, b_q * BL : (b_q + 1) * BL],
                    rhs=accr_q[:],
                    start=(b_q == 0),
                    stop=(b_q == BL - 1),
                )
                if b_q == BL - 1:
                    sl = slice(st_q * ST, (st_q + 1) * ST)
                    nc.vector.tensor_add(scores[:, sl], sc_ps_q[:, :], ma_sb[:, sl])
                    # masked softmax, pipelined: exp + partial row-sum for
                    # this tile run under the remaining main loop
                    nc.scalar.activation(e_sb[:, sl], scores[:, sl], Exp)
                    nc.vector.reduce_sum(
                        sums[:, st_q : st_q + 1],
                        e_sb[:, sl],
                        axis=mybir.AxisListType.X,
                    )

        sc_tiles = [psvdot.tile([BL, ST], f32, name=f"sc_ps{st}", tag="sc_ps") for st in range(nst)]
        for st in range(nst):
            sc_ps = sc_tiles[st]
            for b in range(BL):
                if st == 0 and b == 0:
                    kt = kt0
                else:
                    kt = keysp.tile([128, 2, KB, ST], f8)
                    nc.sync.dma_start(
                        kt[:], keysT_r[:, :, :, b, st * ST : (st + 1) * ST]
                    )
                acc = accp.tile([128, ST], f32)
                accr = accrp.tile([128, ST], f32r)
                for j in range(JB):
                    if st == 0 and b == 0:
                        # just-in-time w1q: emitted into the PE stream right
                        # before the main group that its tanh will need
                        emit_w1q(j)
                    ps = psmain.tile([128, ST], f32)
                    for kk in range(KP):
                        # DoubleRow pair: 256-dim contraction per instr; the
                        # hi and lo passes share the stationary weight pair.
                        # The last pair skips its lo correction: the keys
                        # quantization error of 2/8 k-blocks is negligible
                        # (simulated rel err 0.0162 vs 0.0163 fully corrected)
                        # and it saves 1/8 of the main matmul instructions.
                        w3 = w2_sbj[j][:, kk * 256 : (kk + 1) * 256].rearrange(
                            "p (two m) -> p two m", two=2
                        )
                        last = kk == KP - 1
                        nc.tensor.matmul(
                            ps[:],
                            lhsT=w3,
                            rhs=kt[:, 0, 2 * kk : 2 * kk + 2, :],
                            start=(kk == 0),
                            stop=last,
                            perf_mode=DR,
                        )
                        if not last:
                            nc.tensor.matmul(
                                ps[:],
                                lhsT=w3,
                                rhs=kt[:, 1, 2 * kk : 2 * kk + 2, :],
                                start=False,
                                stop=False,
                                perf_mode=DR,
                            )
                    groups_done += 1
                    flush_ones(2)
                    comb = combp.tile([128, ST], f32)
                    nc.scalar.activation(
                        comb[:], ps[:], Tanh, bias=w1qj[j][:, b : b + 1]
                    )
                    # acc = (comb * v_j) + acc fused on VectorE
                    if j == 0:
                        nc.vector.tensor_scalar_mul(acc[:], comb[:], v_sb[:, 0:1])
                    elif j == JB - 1:
                        nc.vector.scalar_tensor_tensor(
                            accr[:], comb[:], v_sb[:, j : j + 1], acc[:], Mult, Add
                        )
                    else:
                        nc.vector.scalar_tensor_tensor(
                            acc[:], comb[:], v_sb[:, j : j + 1], acc[:], Mult, Add
                        )
                ones_queue.append((sc_ps, accr, b, st, groups_done))
        flush_ones(0)

        # --- softmax tail: combine partial sums, scale, write out -----------
        # scores are bounded by sum|v| (~27), so exp cannot overflow f32 and
        # the max subtraction is unnecessary; masked/padded entries are
        # exp(-1e30)=0. exp + partial sums already ran per tile above.
        sm = singles.tile([BL, 1], f32)
        nc.vector.reduce_sum(sm[:], sums[:, 0:nst], axis=mybir.AxisListType.X)
        rs = singles.tile([BL, 1], f32)
        nc.vector.reciprocal(rs[:], sm[:])
        o_sb = singles.tile([BL, sp], f32)
        H = sp // 2
        for h in range(2):
            nc.vector.tensor_scalar_mul(
                o_sb[:, h * H : (h + 1) * H], e_sb[:, h * H : (h + 1) * H], rs[:, 0:1]
            )
            nc.sync.dma_start(out[:, h * H : (h + 1) * H], o_sb[:, h * H : (h + 1) * H])

    nc.compile()
    return nc


def _get_nc(nst):
    key = ("nc", nst)
    if key not in _cache:
        _cache[key] = _build_nc(nst)
    return _cache[key]


def _weight_blocks(W):
    # [j, p, k*128+ai] = W[j*128+ai, k*128+p]
    return np.ascontiguousarray(
        W.reshape(JB, 128, KB, 128).transpose(0, 3, 2, 1).reshape(JB, 128, KB * 128)
    )


def _make_in_maps(query, keys, mask, W1, W2, v, sp):
    query = np.asarray(query, dtype=np.float32)
    keys = np.asarray(keys, dtype=np.float32)
    mask = np.asarray(mask)
    W1 = np.asarray(W1, dtype=np.float32)
    W2 = np.asarray(W2, dtype=np.float32)
    v = np.asarray(v, dtype=np.float32)

    import ml_dtypes

    f8np = ml_dtypes.float8_e4m3  # TRN float8e4 (max normal 240)
    w2t = _weight_blocks(W2).astype(f8np)
    w1t = _weight_blocks(W1).astype(ml_dtypes.bfloat16)
    vcol = np.ascontiguousarray(v.reshape(JB, 128).T)  # [p, j]
    onesz = np.zeros((128, BL, BL), dtype=np.float32)
    for b in range(BL):
        onesz[:, b, b] = 1.0
    onesz = np.ascontiguousarray(onesz.reshape(128, BL * BL))

    in_maps = []
    idx_all = []
    for c in range(NCORES):
        # mask compaction: gather only active key columns, pad to sp
        kc = np.zeros((BL, sp, D), dtype=np.float32)
        maskadd_c = np.full((BL, sp), -MASK_NEG, dtype=np.float32)
        idx_core = []
        for bl in range(BL):
            gb = c * BL + bl
            idx = np.flatnonzero(mask[gb])
            kc[bl, : len(idx)] = keys[gb][idx]
            maskadd_c[bl, : len(idx)] = 0.0
            idx_core.append(idx)
        idx_all.append(idx_core)
        keysT_c = np.ascontiguousarray(kc.transpose(2, 0, 1))  # (D, BL, sp)
        khi = keysT_c.astype(f8np)
        klo = (keysT_c - khi.astype(np.float32)).astype(f8np)
        keysT8_c = np.stack([khi, klo], axis=0)  # (2, D, BL, sp)
        qT_c = np.ascontiguousarray(
            query[c * BL : (c + 1) * BL].T.reshape(KB, 128, BL).transpose(1, 0, 2)
        )  # (128, KB, BL)
        qTb_c = qT_c.astype(ml_dtypes.bfloat16)
        in_maps.append(
            {
                "keysT8": keysT8_c,
                "w2t": w2t,
                "w1t": w1t,
                "qT": qT_c,
                "qTb": qTb_c,
                "vcol": vcol,
                "onesz": onesz,
                "maskadd": maskadd_c,
            }
        )
    return in_maps, idx_all


def kernel(query, keys, mask, W1, W2, v):
    from concourse.bass_utils import run_bass_kernel_spmd

    mask_np = np.asarray(mask)
    n_max = int(mask_np.sum(axis=1).max())
    nst = max(1, -(-n_max // ST))  # ceil
    sp = nst * ST

    nc = _get_nc(nst)
    in_maps, idx_all = _make_in_maps(query, keys, mask_np, W1, W2, v, sp)
    res = run_bass_kernel_spmd(nc, in_maps, core_ids=list(range(NCORES)))
    _cache["last_results"] = res

    out = np.zeros((B, S), dtype=np.float32)
    for c in range(NCORES):
        oc = res.results[c]["out"]
        for bl in range(BL):
            idx = idx_all[c][bl]
            out[c * BL + bl, idx] = oc[bl, : len(idx)]
    return out
